# revision 15
# baseline (speedup 1.0000x reference)
"""Trainium2 Bass kernel for the MoE-routing module.

Computation (B=32768, D=1024, H=512, F=100, E=16, K=2):
    h   = relu(x @ W_shared + b_shared)                  [B, H]
    a   = relu(einsum('bh,ehf', h, W1) + b1)             [B, E, F]
    o   = einsum('bef,efo', a, W2) + b2                  [B, E, 1]
    out = mean over the K routed experts of o[b, send_to[idx[b]]]

Strategy: host sorts tokens by head id and shards the sorted batch over the
8 cores (4096 tokens each, perfectly balanced).  A sorted 4096-token window
only routes to a handful of consecutive experts, so each core gets just the
expert slices it needs (EC slots, adaptively >= actual need; EC=16 degrades
to the dense all-expert kernel).  Routing is folded into a host-computed
per-slot mask M[j, b], so the device computes
    out[b] = sum_j o_local[b, j] * M[j, b]
with three matmul stages, features on SBUF partitions throughout:
  M1: hT[h, t]  = relu(W_shared.T @ xT)         lhsT = W_shared tiles
  M2: aT[f', t] = relu(W1sel.T @ hT)            f' = j*F + f  (EC*F wide)
  M3: c[j, t]   = W2sel.T @ aT                  W2sel block-diagonal
  sel: out[t]   = ones.T @ (c * mask)           1-partition result row
All matmuls run as float32r (full-rate fp32 mode, ~1e-4 rel err).
"""

import os

import numpy as np

import concourse.mybir as mybir
from concourse import bacc
from concourse.bass_utils import run_bass_kernel_spmd
from concourse.tile import TileContext

B, D, H, F, E, TOPK = 32768, 1024, 512, 100, 16, 2
N_CORES = 8
BL = B // N_CORES          # tokens per core
CHUNK = 512                # tokens per device-side tile loop
N_CHUNKS = BL // CHUNK
MH = H // 128              # M1 output tiles
KD = D // 128              # M1 contraction tiles
KH = H // 128              # M2 contraction tiles
EC_MIN = 5                 # minimum expert slots per core

# Compute dtype for the matmul stages: "float32", "float32r", or "bfloat16"
COMPUTE_DT = os.environ.get("KERNEL_DT", "float32r")

_FP32 = mybir.dt.float32
_cache = {}


def _np_in_dtype():
    import ml_dtypes

    return ml_dtypes.bfloat16 if COMPUTE_DT == "bfloat16" else np.float32


def _build_nc(ec):
    """Build the SPMD program for EC expert slots per core."""
    CDT = getattr(mybir.dt, COMPUTE_DT)
    SDT = mybir.dt.bfloat16 if COMPUTE_DT == "bfloat16" else mybir.dt.float32
    EF = ec * F                    # local expert-concat width
    KT3 = (EF + 127) // 128        # M2 output tiles / M3 contraction tiles
    EF_PAD = KT3 * 128             # w1sel zero-padded so all tiles are full
    NB = MH + KT3 + 1              # packed bias columns

    nc = bacc.Bacc("TRN2", target_bir_lowering=False, num_devices=N_CORES)

    xT_d = nc.declare_dram_parameter("xT", [N_CHUNKS, D, CHUNK], CDT, isOutput=False)
    mask_d = nc.declare_dram_parameter("mask", [33, BL], _FP32, isOutput=False)
    wsh_d = nc.declare_dram_parameter("wsh", [D, H], CDT, isOutput=False)
    w1c_d = nc.declare_dram_parameter("w1c", [H, EF_PAD], CDT, isOutput=False)
    w2bd_d = nc.declare_dram_parameter("w2bd", [128, KT3 * ec], CDT, isOutput=False)
    bias_d = nc.declare_dram_parameter("biases", [128, NB], _FP32, isOutput=False)
    out_d = nc.declare_dram_parameter("out", [BL], _FP32, isOutput=True)

    relu = mybir.ActivationFunctionType.Relu

    with TileContext(nc) as tc:
        with (
            tc.tile_pool(name="weights", bufs=1) as wpool,
            tc.tile_pool(name="xin", bufs=3) as xpool,
            tc.tile_pool(name="mid", bufs=2) as midpool,
            tc.tile_pool(name="small", bufs=2) as spool,
            tc.tile_pool(name="ps_h", bufs=4, space="PSUM") as ps_h,
            tc.tile_pool(name="ps_a", bufs=2, space="PSUM") as ps_a,
            tc.tile_pool(name="ps_c", bufs=1, space="PSUM") as ps_c,
            tc.tile_pool(name="ps_o", bufs=1, space="PSUM") as ps_o,
        ):
            # ---- startup-critical loads: wsh + chunk-0 x, split per k-tile
            # and interleaved across both HWDGE queues (Sync + Activation)
            # so M1 of chunk 0 can start after the first ~512KB lands.
            wsh_view = wsh_d.rearrange("(o p) h -> p o h", p=128)
            wsh_sb = wpool.tile([128, KD, H], CDT)
            xt0 = xpool.tile([128, KD, CHUNK], CDT, tag="xt")
            xt0_view = xT_d[0].rearrange("(o p) t -> p o t", p=128)
            with tc.high_priority():
                for k in range(KD):
                    qa = nc.sync if k % 2 == 0 else nc.scalar
                    qb = nc.scalar if k % 2 == 0 else nc.sync
                    qa.dma_start(wsh_sb[:, k], wsh_view[:, k])
                    qb.dma_start(xt0[:, k], xt0_view[:, k])

            xts, masks = [xt0], []
            for c in range(N_CHUNKS):
                if c > 0:
                    xt = xpool.tile([128, KD, CHUNK], CDT, tag="xt")
                    xv = xT_d[c].rearrange("(o p) t -> p o t", p=128)
                    nc.scalar.dma_start(xt[:, : KD // 2], xv[:, : KD // 2])
                    nc.sync.dma_start(xt[:, KD // 2 :], xv[:, KD // 2 :])
                    xts.append(xt)
                mask_sb = spool.tile([33, CHUNK], _FP32, tag="mask")
                nc.scalar.dma_start(mask_sb[:], mask_d[:, c * CHUNK : (c + 1) * CHUNK])
                masks.append(mask_sb)
                if c == 0:
                    w1c_sb = wpool.tile([128, KH, EF_PAD], CDT)
                    w1c_view = w1c_d.rearrange("(o p) f -> p o f", p=128)
                    nc.sync.dma_start(w1c_sb[:, : KH // 2], w1c_view[:, : KH // 2])
                    nc.scalar.dma_start(w1c_sb[:, KH // 2 :], w1c_view[:, KH // 2 :])
                    w2bd_sb = wpool.tile([128, KT3 * ec], CDT)
                    nc.sync.dma_start(w2bd_sb[:], w2bd_d[:])
                    bias_sb = wpool.tile([128, NB], _FP32)
                    nc.sync.dma_start(bias_sb[:], bias_d[:])
                    ones_sb = wpool.tile([ec, 1], CDT)
                    nc.vector.memset(ones_sb[:].bitcast(mybir.dt.float32), 1.0)

            for c in range(N_CHUNKS):
                t0 = c * CHUNK
                xt = xts[c]
                mask_sb = masks[c]

                # ---- M1: hT = relu(W_shared.T @ xT + b) ----
                # chunk 0 runs k-outer so matmuls start as soon as the first
                # split DMA pieces land; later chunks are fully prefetched.
                hT = midpool.tile([128, MH, CHUNK], CDT, tag="hT")
                if c == 0:
                    phs = [ps_h.tile([128, CHUNK], _FP32, tag="ps_h", name=f"ph{m}") for m in range(MH)]
                    for k in range(KD):
                        for m in range(MH):
                            nc.tensor.matmul(
                                phs[m][:],
                                lhsT=wsh_sb[:, k, m * 128 : (m + 1) * 128],
                                rhs=xt[:, k, :],
                                start=(k == 0),
                                stop=(k == KD - 1),
                            )
                    for m in range(MH):
                        nc.scalar.activation(
                            hT[:, m, :], phs[m][:], relu, bias=bias_sb[:, m : m + 1]
                        )
                else:
                    for m in range(MH):
                        ph = ps_h.tile([128, CHUNK], _FP32, tag="ps_h")
                        for k in range(KD):
                            nc.tensor.matmul(
                                ph[:],
                                lhsT=wsh_sb[:, k, m * 128 : (m + 1) * 128],
                                rhs=xt[:, k, :],
                                start=(k == 0),
                                stop=(k == KD - 1),
                            )
                        nc.scalar.activation(
                            hT[:, m, :], ph[:], relu, bias=bias_sb[:, m : m + 1]
                        )

                # ---- M2: aT = relu(W1sel.T @ hT + b1) ----
                aT = midpool.tile([128, KT3, CHUNK], CDT, tag="aT")
                for m in range(KT3):
                    f0 = m * 128
                    pa = ps_a.tile([128, CHUNK], _FP32, tag="ps_a")
                    for k in range(KH):
                        nc.tensor.matmul(
                            pa[:],
                            lhsT=w1c_sb[:, k, f0 : f0 + 128],
                            rhs=hT[:, k, :],
                            start=(k == 0),
                            stop=(k == KH - 1),
                        )
                    nc.scalar.activation(
                        aT[:, m, :], pa[:], relu,
                        bias=bias_sb[:, MH + m : MH + m + 1],
                    )

                # ---- M3: c = W2sel.T @ aT  (block-diag W2) ----
                pc = ps_c.tile([ec, CHUNK], _FP32, tag="ps_c")
                for k in range(KT3):
                    nc.tensor.matmul(
                        pc[:],
                        lhsT=w2bd_sb[:, k * ec : (k + 1) * ec],
                        rhs=aT[:, k, :],
                        start=(k == 0),
                        stop=(k == KT3 - 1),
                    )

                # ---- select: out = ones.T @ (c * mask) + btok ----
                msel = spool.tile([ec, CHUNK], CDT, tag="msel")
                nc.vector.tensor_mul(msel[:], pc[:], mask_sb[:ec, :])
                po = ps_o.tile([1, CHUNK], _FP32, tag="ps_o")
                nc.tensor.matmul(po[:], lhsT=ones_sb[:], rhs=msel[:], start=True, stop=True)
                ot = spool.tile([1, CHUNK], _FP32, tag="ot")
                nc.vector.tensor_add(ot[:], po[:], mask_sb[32:33, :])
                nc.gpsimd.dma_start(out_d[t0 : t0 + CHUNK].rearrange("(o t) -> o t", o=1), ot[:])

    nc.compile()
    return nc


def get_nc(ec):
    key = (COMPUTE_DT, ec)
    if key not in _cache:
        _cache[key] = _build_nc(ec)
    return _cache[key]


def prepare(inputs):
    """Host-side routing/sorting/sharding. Returns (ec, in_maps, perm)."""
    np_dt = _np_in_dtype()
    x = np.asarray(inputs["x"], dtype=np.float32)
    idx = np.asarray(inputs["idx"]).astype(np.int64).reshape(B)
    W_shared = np.asarray(inputs["W_shared"], dtype=np.float32)
    b_shared = np.asarray(inputs["b_shared"], dtype=np.float32).reshape(H)
    W1 = np.asarray(inputs["W1"], dtype=np.float32)
    b1 = np.asarray(inputs["b1"], dtype=np.float32).reshape(E, F)
    W2 = np.asarray(inputs["W2"], dtype=np.float32).reshape(E, F)
    b2 = np.asarray(inputs["b2"], dtype=np.float32).reshape(E)
    send_to = np.asarray(inputs["send_to"]).astype(np.int64)

    perm = np.argsort(idx, kind="stable")
    idx_s = idx[perm]
    routes_s = send_to[idx_s]                      # [B, K] sorted routes
    x_s = x[perm]                                  # [B, D]

    # per-core expert lists
    expert_lists = []
    for c in range(N_CORES):
        sl = slice(c * BL, (c + 1) * BL)
        expert_lists.append(np.unique(routes_s[sl]))
    ec = max(EC_MIN, max(len(el) for el in expert_lists))
    ec = min(ec, E)

    wsh = np.ascontiguousarray(W_shared).astype(np_dt)
    EF = ec * F
    KT3 = (EF + 127) // 128
    EF_PAD = KT3 * 128
    NB = MH + KT3 + 1

    in_maps = []
    for c in range(N_CORES):
        sl = slice(c * BL, (c + 1) * BL)
        el = expert_lists[c]
        # local slot tables (pad slots use sentinel -1: zero weights, no mask)
        slots = np.full(ec, -1, dtype=np.int64)
        slots[: len(el)] = el

        # mask[j, b] = (1/K) * count of slots[j] among routes of token b
        r = routes_s[sl]                            # [BL, K]
        mask = np.zeros((33, BL), dtype=np.float32)
        for k in range(r.shape[1]):
            hit = slots[:, None] == r[None, :, k]   # [ec, BL]
            mask[:ec] += hit.astype(np.float32) / r.shape[1]
        mask[32] = b2[r].mean(axis=1)               # routed-b2 mean per token

        w1sel = np.zeros((H, EF_PAD), dtype=np.float32)
        b1sel = np.zeros(EF_PAD, dtype=np.float32)
        w2full = np.zeros((EF_PAD, ec), dtype=np.float32)
        for j, e in enumerate(slots):
            if e < 0:
                continue
            w1sel[:, j * F : (j + 1) * F] = W1[e]
            b1sel[j * F : (j + 1) * F] = b1[e]
            w2full[j * F : (j + 1) * F, j] = W2[e]
        w2bd = np.ascontiguousarray(
            w2full.reshape(KT3, 128, ec).transpose(1, 0, 2).reshape(128, KT3 * ec)
        ).astype(np_dt)

        biases = np.zeros((128, NB), dtype=np.float32)
        biases[:, :MH] = b_shared.reshape(MH, 128).T
        biases[:, MH : MH + KT3] = b1sel.reshape(KT3, 128).T
        biases[:ec, MH + KT3] = b2[np.maximum(slots, 0)] * (slots >= 0)

        xT = np.ascontiguousarray(
            x_s[sl].reshape(N_CHUNKS, CHUNK, D).transpose(0, 2, 1)
        ).astype(np_dt)

        in_maps.append(
            {
                "xT": xT,
                "mask": mask,
                "wsh": wsh,
                "w1c": w1sel.astype(np_dt),
                "w2bd": w2bd,
                "biases": biases,
            }
        )
    return ec, in_maps, perm


def kernel(**inputs) -> np.ndarray:
    ec, in_maps, perm = prepare(inputs)
    nc = get_nc(ec)
    res = run_bass_kernel_spmd(nc, in_maps, list(range(N_CORES)))
    out_sorted = np.concatenate([res.results[c]["out"] for c in range(N_CORES)])
    out = np.empty(B, dtype=np.float32)
    out[perm] = out_sorted
    return out.reshape(B, 1)
